# revision 21
# baseline (speedup 1.0000x reference)
"""Single-head attention kernel for Trainium2 (8 NeuronCores, SPMD).

Problem: x[4,4096,1024] f32, padding_mask[4,1,4096] i32, Wk/Wq/Wv[64,1024] f32.
  k/q/v = x @ W.T ; wei = softmax(mask(q k^T / 8)) ; out = wei @ v  -> [4,4096,64]

Sharding: core c = (b = c//2, half = c%2). The host rotates x[b] (and the key
mask) so this core's 2048 queries are always rows 0:2048 -- attention is
permutation-invariant over keys when the key mask rotates identically.  Each
core computes k/v for all 4096 keys and q for its local half, returning
out[2048, 64].  No cross-core exchange.

Design (vs the 298us v3 baseline; now ~134us):
  - x is transposed on the HOST: xT [128, 8, 4096] bf16 DMAs straight into
    SBUF (kills 8 serialized on-device DMA-XBAR transposes).  First chunk is
    split fine so the first projection matmul starts ~11.5us in.
  - Stationary is [wk | wq]: k lands at PSUM partitions 0:64 and engine-copies
    straight into kT3; q (g0 only) takes a small partition hop instead.
  - g1 (non-local tokens) needs k only: col-packed pairs (tile_position
    (0,0)/(0,64)) run two 64-wide matmuls concurrently in the PE array.
  - One LDWEIGHTS per stationary change (explicit ldweights + noload
    matmuls), not one per matmul.
  - v transposes batched: 4 XBAR transposes of [64,512] per group into a
    CONTIGUOUS staging tile (a strided transpose dest silently corrupts!),
    then strided engine copies into v_sb [key, 65] (65th col = ones so the
    PV matmul accumulates softmax denominators for free).
  - Phase 2 software pipeline, kc-major: per kc the PE runs 4 score matmuls
    (sT[128k,512q] = kT^T qT) into 2x2-bank PSUM groups, ACT does TWO wide
    exps ([128,1024] each, bias -1e5*(1-mask) underflows masked keys to 0,
    bf16 out), and PV (oT[65,512] += v_ext^T exp) lags one kc.  ACT is the
    bottleneck at ~2.0us/kc; the PE hides under it.  kc0's scores/exp are
    emitted inside phase 1 so the phase boundary never idles the PE (an idle
    boundary re-throttles the HAM clock gate to 1.2 GHz for the next ~8 kc).
  - Epilogue: one batched XBAR transpose of oT, reciprocal+mask fused into a
    scale vector, per-tt multiplies split across vector/scalar/gpsimd, output
    DMA split in halves.
"""

import sys

if "/opt/trn_rl_repo" not in sys.path:
    sys.path.insert(0, "/opt/trn_rl_repo")

import numpy as np
import ml_dtypes

import concourse.bass as bass
import concourse.mybir as mybir
import concourse.tile as tile
from concourse import bacc
from concourse.bass_utils import run_bass_kernel_spmd

F32 = mybir.dt.float32
BF16 = mybir.dt.bfloat16

DEBUG = False

T = 4096
TL = 2048
C = 1024
H = 64
NCC = 8
NKC = 32
NQB = 4
NTT = TL // 128   # 16
NEG = -1.0e5


def mm_noload(nc, out, lhsT, rhs, start=True, stop=True, tile_position=None,
              tile_size=(128, 128)):
    te = nc.tensor
    keep = {0}
    ifmap_ap = te.lower_ap(rhs.opt(keep), opt=False)
    weights_ap = te.lower_ap(lhsT.opt(keep), opt=False, for_matmul_weights=True)
    out_ap = te.lower_ap(out)
    if tile_position is None:
        tile_position = (rhs.base_partition(), out.base_partition())
    return te.add_instruction(
        mybir.InstMatmult(
            name=f"I-{nc.next_id()}",
            replication_resolution=0,
            replication_shift_amnt=0,
            replication_num_rows=0,
            start_tensor_calc=start,
            stop_tensor_calc=stop,
            ins=[ifmap_ap, weights_ap],
            outs=[out_ap],
            perf_mode=None,
            is_transpose=None,
            tile_position=tile_position,
            tile_size=tile_size,
            ldweights=False,
        )
    )


def _copy(nc, eng, out, in_):
    if eng is nc.scalar:
        eng.copy(out, in_)
    else:
        eng.tensor_copy(out, in_)


def _v_copies(nc, g, vt_ps, vT_sb, v_stg, v_sb):
    engines = [nc.vector, nc.scalar]
    for tl in range(4):
        r0 = (tl % 2) * 64
        c0 = (tl // 2) * 512
        _copy(nc, engines[(tl + 1) % 2], vT_sb[r0:r0 + 64, c0:c0 + 512],
              vt_ps[tl // 2][r0:r0 + 64, :])
    # batched v transposes: keys for (r0,c0) block are tb*512:(tb+1)*512 with
    # tb = 4g+tl, i.e. kc chunks 4*tb..4*tb+4.  The XBAR writes the transposed
    # block CONTIGUOUSLY -- a strided dest AP silently corrupts -- so land in
    # contiguous v_stg, then strided engine copies into v_sb (66-stride).
    for tl in range(4):
        r0 = (tl % 2) * 64
        c0 = (tl // 2) * 512
        nc.sync.dma_start_transpose(
            v_stg[:, g, 4 * tl:4 * tl + 4, :], vT_sb[r0:r0 + 64, c0:c0 + 512]
        )
    for hh in range(2):
        _copy(nc, engines[hh], v_sb[:, g * 16 + 8 * hh:g * 16 + 8 * hh + 8, 0:64],
              v_stg[:, g, 8 * hh:8 * hh + 8, :])


def _g0_copies(nc, kq_pair, vt_ps, qstage, qT3, kT3, vT_sb, v_stg, v_sb):
    engines = [nc.vector, nc.scalar]
    for tl in range(4):
        kq = kq_pair[tl // 2][:, tl % 2, :]
        # k at partitions 0:64 -> straight into kT3 (cast f32->bf16)
        _copy(nc, engines[tl % 2], kT3[:, 4 * tl:4 * tl + 4, :],
              kq[0:64, :].rearrange("p (kc f) -> p kc f", kc=4))
        # local queries: hop partitions 64:128 -> 0:64
        _copy(nc, engines[(tl + 1) % 2], qstage[64:128, tl, :], kq[64:128, :])
        nc.gpsimd.dma_start(out=qT3[:, tl, :], in_=qstage[64:128, tl, :])
    _v_copies(nc, 0, vt_ps, vT_sb, v_stg, v_sb)


def _g1_k_copies(nc, kq_g1, qstage, kT3):
    # kT3-producing copies FIRST on both engines: phase 2's pulled-ahead
    # LDWEIGHTS stall the PE in retire order if these land late.
    engines = [nc.vector, nc.scalar]
    for pair in range(2):
        tb_e = 4 + 2 * pair
        _copy(nc, engines[pair % 2], kT3[:, 4 * tb_e:4 * tb_e + 4, :],
              kq_g1[0:64, pair, :].rearrange("p (kc f) -> p kc f", kc=4))
        # odd-tb k sits at partitions 64:128 -> stage + hop down
        _copy(nc, engines[(pair + 1) % 2], qstage[64:128, 2 * pair, :],
              kq_g1[64:128, pair, :])
    for pair in range(2):
        tb_o = 5 + 2 * pair
        nc.gpsimd.dma_start(
            out=kT3[:, 4 * tb_o:4 * tb_o + 4, :],
            in_=qstage[64:128, 2 * pair, :].rearrange("p (kc f) -> p kc f", kc=4),
        )


def build_nc():
    nc = bacc.Bacc("TRN2", target_bir_lowering=False, debug=False, num_devices=8)

    xt_d = nc.dram_tensor("xt", [128, NCC, T], BF16, kind="ExternalInput")
    wkq_d = nc.dram_tensor("wkq", [128, NCC, 128], BF16, kind="ExternalInput")
    wv_d = nc.dram_tensor("wv", [128, NCC, H], BF16, kind="ExternalInput")
    nbias_d = nc.dram_tensor("nbias", [128, NKC], F32, kind="ExternalInput")
    maskq_d = nc.dram_tensor("maskq", [128, NTT], F32, kind="ExternalInput")
    out_d = nc.dram_tensor("out", [TL, H], F32, kind="ExternalOutput")
    dbg = {}
    if DEBUG:
        dbg["kT"] = nc.dram_tensor("dbg_kT", [64, NKC * 128], BF16, kind="ExternalOutput")
        dbg["qT"] = nc.dram_tensor("dbg_qT", [64, NQB * 512], BF16, kind="ExternalOutput")
        dbg["v"] = nc.dram_tensor("dbg_v", [128, NKC * 66], BF16, kind="ExternalOutput")

    with tile.TileContext(nc) as tc:
        with (
            tc.tile_pool(name="const", bufs=1) as const,
            tc.tile_pool(name="persist", bufs=1) as persist,
            tc.tile_pool(name="expp", bufs=6) as expp,
            tc.tile_pool(name="osb", bufs=1) as osb,
            tc.tile_pool(name="small", bufs=4) as small,
            tc.tile_pool(name="PS", bufs=2, space=bass.MemorySpace.PSUM) as PS,
            tc.tile_pool(name="PO", bufs=4, space=bass.MemorySpace.PSUM) as PO,
        ):
            _emit(nc, const, persist, expp, osb, small, PS, PO,
                  xt_d, wkq_d, wv_d, nbias_d, maskq_d, out_d, dbg)

    nc.compile()
    return nc


def _emit(nc, const, persist, expp, osb, small, PS, PO,
          xt_d, wkq_d, wv_d, nbias_d, maskq_d, out_d, dbg=None):
    # ---------------- constants / persistent tiles ----------------
    wkq_sb = const.tile([128, NCC, 128], BF16)
    wv_sb = const.tile([128, NCC, H], BF16)
    nbias_sb = const.tile([128, NKC], F32)
    maskq_sb = const.tile([128, NTT], F32)
    # wkq gates the first matmul -- gpsimd queue is free earliest
    nc.gpsimd.dma_start(out=wkq_sb, in_=wkq_d.ap())
    nc.gpsimd.dma_start(out=wv_sb, in_=wv_d.ap())
    nc.gpsimd.dma_start(out=nbias_sb, in_=nbias_d.ap())
    nc.gpsimd.dma_start(out=maskq_sb, in_=maskq_d.ap())

    xT_sb = persist.tile([128, NCC, T], BF16)
    kT3 = persist.tile([64, NKC, 128], BF16)
    qT3 = persist.tile([64, NQB, 512], BF16)
    qstage = persist.tile([128, NQB, 512], BF16)
    v_sb = persist.tile([128, NKC, 66], BF16)       # [key, 65(+pad)]
    v_stg = persist.tile([128, 2, 16, 64], BF16)    # contiguous xbar dest
    vT_sb = persist.tile([128, 2 * 512], BF16)      # vT staging, parity rows
    oTT = persist.tile([128, NTT, 80], BF16)
    out_acc = persist.tile([128, NTT, H], F32)

    ones_sb = const.tile([128, NKC], BF16)
    nc.gpsimd.memset(ones_sb, 1.0)
    nc.gpsimd.tensor_copy(v_sb[:, :, 64], ones_sb)

    # ---------------- 1) x^T loads (host pre-transposed), g0 halves first --
    # First chunk split per-tl so the first projection matmul starts on a
    # 128KB transfer instead of waiting out a full 512KB one.
    for tl in range(4):
        q = nc.sync if tl % 2 == 0 else nc.scalar
        q.dma_start(
            out=xT_sb[:, 0, tl * 512:(tl + 1) * 512],
            in_=xt_d.ap()[:, 0, tl * 512:(tl + 1) * 512],
        )
    qi = 0
    for g in range(2):
        for cc in range(NCC):
            if g == 0 and cc == 0:
                continue
            q = nc.sync if qi % 2 == 0 else nc.scalar
            qi += 1
            q.dma_start(
                out=xT_sb[:, cc, g * TL:(g + 1) * TL],
                in_=xt_d.ap()[:, cc, g * TL:(g + 1) * TL],
            )

    # ---------------- 2) projections ----------------
    # g0: full [wk|wq] stationary (queries are local rows 0:2048)
    kq_pair = [PS.tile([128, 2, 512], F32, tag="s", name="kq")
               for _ in range(2)]
    vt_ps = [PO.tile([128, 512], F32, tag="o", name="vt") for _ in range(2)]
    for cc in range(NCC):
        first, last = cc == 0, cc == NCC - 1
        nc.tensor.ldweights(wkq_sb[:, cc, :])
        for tl in range(4):
            mm_noload(
                nc, kq_pair[tl // 2][:, tl % 2, :],
                wkq_sb[:, cc, :],
                xT_sb[:, cc, tl * 512:(tl + 1) * 512],
                start=first, stop=last,
            )
        for tl in range(4):
            r0 = (tl % 2) * 64
            nc.tensor.matmul(
                vt_ps[tl // 2][r0:r0 + 64, :],
                wv_sb[:, cc, :],
                xT_sb[:, cc, tl * 512:(tl + 1) * 512],
                start=first, stop=last,
            )
    _g0_copies(nc, kq_pair, vt_ps, qstage, qT3, kT3, vT_sb, v_stg, v_sb)

    # g1: k-only, col-packed -- tb pairs run concurrently in the PE array
    # (tb even -> array cols 0:64 / psum partitions 0:64, tb odd -> 64:128)
    kq_g1 = PS.tile([128, 2, 512], F32, tag="s", name="kq1")
    vt_ps1 = [PO.tile([128, 512], F32, tag="o", name="vt1") for _ in range(2)]
    # kq pass FIRST, v pass second: kq_g1's PSUM slot frees (via its copies)
    # while the v matmuls still run, so phase 2's kc0 score banks are ready
    # the moment g1 ends -- no PE hole at the phase boundary, HAM stays warm.
    for cc in range(NCC):
        first, last = cc == 0, cc == NCC - 1
        wk = wkq_sb[:, cc, 0:64]
        nc.tensor.ldweights(wk, tile_position=(0, 0))
        nc.tensor.ldweights(wk, tile_position=(0, 64))
        for pair in range(2):
            tb_e, tb_o = 4 + 2 * pair, 5 + 2 * pair
            mm_noload(
                nc, kq_g1[0:64, pair, :], wk,
                xT_sb[:, cc, tb_e * 512:(tb_e + 1) * 512],
                start=first, stop=last,
                tile_position=(0, 0), tile_size=(128, 64),
            )
            mm_noload(
                nc, kq_g1[64:128, pair, :], wk,
                xT_sb[:, cc, tb_o * 512:(tb_o + 1) * 512],
                start=first, stop=last,
                tile_position=(0, 64), tile_size=(128, 64),
            )
    _g1_k_copies(nc, kq_g1, qstage, kT3)

    # phase-2 scores+exp for kc0..2 emitted HERE: keeps the PE/ACT queues
    # full while g1's v chain (matmuls -> copies -> transposes) drains, so
    # the phase-boundary sync resolves under queued work instead of idling
    # the PE (an idle boundary re-throttles HAM to 1.2 GHz for ~8 kc).
    def emit_scores(kc):
        es = []
        nc.tensor.ldweights(kT3[:, kc, :])
        for grp in range(2):
            s = PS.tile([128, 2, 512], F32, tag="s", name="sT")
            for j in range(2):
                mm_noload(
                    nc, s[:, j, :], kT3[:, kc, :], qT3[:, 2 * grp + j, :],
                    start=True, stop=True,
                )
            e = expp.tile([128, 2, 512], BF16, name="exp")
            nc.scalar.activation(
                e, s, mybir.ActivationFunctionType.Exp,
                bias=nbias_sb[:, kc:kc + 1], scale=0.125,
            )
            es.append(e)
        return es

    def emit_pv(pkc, pes):
        nc.tensor.ldweights(v_sb[:, pkc, 0:65])
        for qb in range(NQB):
            mm_noload(
                nc, oT_ps[qb][0:65, :],
                v_sb[:, pkc, 0:65], pes[qb // 2][:, qb % 2, :],
                start=(pkc == 0), stop=(pkc == NKC - 1),
            )

    LAG = 3
    pending = [(kc, emit_scores(kc)) for kc in range(LAG)]

    for cc in range(NCC):
        first, last = cc == 0, cc == NCC - 1
        for tl in range(4):
            tb = 4 + tl
            r0 = (tl % 2) * 64
            nc.tensor.matmul(
                vt_ps1[tl // 2][r0:r0 + 64, :],
                wv_sb[:, cc, :],
                xT_sb[:, cc, tb * 512:(tb + 1) * 512],
                start=first, stop=last,
            )
    _v_copies(nc, 1, vt_ps1, vT_sb, v_stg, v_sb)

    if DEBUG:
        nc.gpsimd.dma_start(out=dbg["kT"].ap(), in_=kT3.rearrange("p a f -> p (a f)"))
        nc.gpsimd.dma_start(out=dbg["qT"].ap(), in_=qT3.rearrange("p a f -> p (a f)"))
        nc.gpsimd.dma_start(out=dbg["v"].ap(), in_=v_sb.rearrange("p a f -> p (a f)"))

    # ---------------- 3) phase 2: kc-major, PV lags scores by LAG kc ------
    oT_ps = [PO.tile([128, 512], F32, tag="o", name=f"oT{i}")
             for i in range(NQB)]
    for kc in range(LAG, NKC):
        pending.append((kc, emit_scores(kc)))
        pkc, pes = pending.pop(0)
        emit_pv(pkc, pes)
    for pkc, pes in pending:
        emit_pv(pkc, pes)

    # ---------------- 4) epilogue ----------------
    # Batched XBAR fold: oTT[p, 4*qb+b, j] = oTs[j, qb, b*128+p], i.e.
    # query = qb*512 + b*128 + p = tt*128 + p with tt = 4*qb+b.
    oTs = osb.tile([80, NQB, 512], BF16)
    engines = [nc.vector, nc.scalar]
    for qb in range(NQB):
        if qb % 2 == 0:
            nc.vector.tensor_copy(oTs[0:65, qb, :], oT_ps[qb][0:65, :])
        else:
            nc.scalar.copy(oTs[0:65, qb, :], oT_ps[qb][0:65, :])
    nc.sync.dma_start_transpose(
        oTT, oTs.rearrange("p a f -> p (a f)")
    )
    recip_all = small.tile([128, NTT], F32)
    scale_all = small.tile([128, NTT], F32)
    nc.vector.reciprocal(
        recip_all, oTT[:, :, 64:65].rearrange("p a one -> p (a one)")
    )
    nc.vector.tensor_tensor(
        scale_all, recip_all, maskq_sb, mybir.AluOpType.mult
    )
    out_dv = out_d.ap().rearrange("(n p) h -> p n h", p=128)
    for half in range(2):
        for i in range(8):
            tt = half * 8 + i
            sc = scale_all[:, tt:tt + 1]
            if i % 3 == 0:
                nc.vector.tensor_scalar(
                    out=out_acc[:, tt, :], in0=oTT[:, tt, 0:64],
                    scalar1=sc, scalar2=None, op0=mybir.AluOpType.mult,
                )
            elif i % 3 == 1:
                nc.scalar.mul(out_acc[:, tt, :], oTT[:, tt, 0:64], sc)
            else:
                nc.gpsimd.tensor_scalar(
                    out=out_acc[:, tt, :], in0=oTT[:, tt, 0:64],
                    scalar1=sc, scalar2=None, op0=mybir.AluOpType.mult,
                )
        nc.gpsimd.dma_start(
            out=out_dv[:, half * 8:(half + 1) * 8, :],
            in_=out_acc[:, half * 8:(half + 1) * 8, :],
        )


_NC_CACHE = None


def _get_nc():
    global _NC_CACHE
    if _NC_CACHE is None:
        _NC_CACHE = build_nc()
    return _NC_CACHE


def make_in_maps(x, padding_mask, Wk, Wq, Wv):
    x = np.asarray(x)
    padding_mask = np.asarray(padding_mask)

    def wt(w):  # [64,1024] -> [128, 8, 64]: wt[p, cc, h] = w[h, cc*128+p]
        return np.ascontiguousarray(
            np.asarray(w).T.reshape(NCC, 128, H).transpose(1, 0, 2)
        )

    wkt, wqt, wvt = wt(Wk), wt(Wq), wt(Wv)
    # stationary [wk | wq] -> psum rows 0:64 = k, 64:128 = q
    wkq = np.concatenate([wkt, wqt], axis=2).astype(ml_dtypes.bfloat16)
    wv = wvt.astype(ml_dtypes.bfloat16)

    in_maps = []
    for core in range(8):
        b, half = core // 2, core % 2
        # rotate keys so this core's queries are always rows 0:2048
        # (attention is permutation-invariant over keys when the key mask
        # is rotated identically)
        q0 = half * TL
        xb = np.roll(x[b], -q0, axis=0).astype(ml_dtypes.bfloat16)
        # host transpose: xt[p, cc, t] = xb[t, cc*128+p]
        xt = np.ascontiguousarray(
            xb.T.reshape(NCC, 128, T).transpose(1, 0, 2)
        )
        m = np.roll(padding_mask[b, 0].astype(np.float32), -q0)
        nbias = np.ascontiguousarray(
            (NEG * (1.0 - m)).reshape(NKC, 128).T
        )
        maskq = np.ascontiguousarray(m[0:TL].reshape(NTT, 128).T)
        in_maps.append({
            "xt": xt, "wkq": wkq, "wv": wv,
            "nbias": nbias, "maskq": maskq,
        })
    return in_maps


def kernel(x, padding_mask, Wk, Wq, Wv):
    nc = _get_nc()
    in_maps = make_in_maps(x, padding_mask, Wk, Wq, Wv)
    res = run_bass_kernel_spmd(nc, in_maps, core_ids=list(range(8)), trace=False)
    B = np.asarray(x).shape[0]
    out = np.empty((B, T, H), dtype=np.float32)
    for c in range(8):
        b, half = c // 2, c % 2
        out[b, half * TL:(half + 1) * TL, :] = res.results[c]["out"]
    return out


# revision 22
# speedup vs baseline: 1.2016x; 1.2016x over previous
"""Single-head attention kernel v5 for Trainium2 (8 NeuronCores, SPMD).

Problem: x[4,4096,1024] f32, padding_mask[4,1,4096] i32, Wk/Wq/Wv[64,1024] f32.
  k/q/v = x @ W.T ; wei = softmax(mask(q k^T / 8)) ; out = wei @ v  -> [4,4096,64]

Sharding: core c = (b = c//2, half = c%2). The host rotates x[b] (and the key
mask) so this core's 2048 queries are always rows 0:2048 -- attention is
permutation-invariant over keys when the key mask rotates identically.  Each
core computes k/v for all 4096 keys and q for its local half, returning
out[2048, 64].  No cross-core exchange.

v5 changes vs v4 (177us):
  - Stationary is [wk | wq]: k lands at PSUM partitions 0:64 -> direct engine
    copy into kT3 (no kstage staging + DMA hop).  q (g0 only) takes the small
    hop (qstage -> DMA down to partitions 0:64) instead.
  - v transposes batched: 4 XBAR transposes of [64,512] per group straight
    into v_sb slices (vs 16 of [64,128] + v_stg + gpsimd copy).  v4's 19us
    phase-1 stall was head-of-line blocking behind these on the sync queue.
  - x DMA split into per-(cc, half) chunks, g0 halves first, so g0
    projections start after ~1.5us of DMA.
  - Epilogue: one batched oT transpose; recip+mask fused into one scale
    vector; per-tt multiplies split across vector/scalar engines.
"""

import sys

if "/opt/trn_rl_repo" not in sys.path:
    sys.path.insert(0, "/opt/trn_rl_repo")

import numpy as np
import ml_dtypes

import concourse.bass as bass
import concourse.mybir as mybir
import concourse.tile as tile
from concourse import bacc
from concourse.bass_utils import run_bass_kernel_spmd

F32 = mybir.dt.float32
BF16 = mybir.dt.bfloat16

DEBUG = False

T = 4096
TL = 2048
C = 1024
H = 64
NCC = 8
NKC = 32
NQB = 4
NTT = TL // 128   # 16
NEG = -1.0e5


def mm_noload(nc, out, lhsT, rhs, start=True, stop=True, tile_position=None,
              tile_size=(128, 128)):
    te = nc.tensor
    keep = {0}
    ifmap_ap = te.lower_ap(rhs.opt(keep), opt=False)
    weights_ap = te.lower_ap(lhsT.opt(keep), opt=False, for_matmul_weights=True)
    out_ap = te.lower_ap(out)
    if tile_position is None:
        tile_position = (rhs.base_partition(), out.base_partition())
    return te.add_instruction(
        mybir.InstMatmult(
            name=f"I-{nc.next_id()}",
            replication_resolution=0,
            replication_shift_amnt=0,
            replication_num_rows=0,
            start_tensor_calc=start,
            stop_tensor_calc=stop,
            ins=[ifmap_ap, weights_ap],
            outs=[out_ap],
            perf_mode=None,
            is_transpose=None,
            tile_position=tile_position,
            tile_size=tile_size,
            ldweights=False,
        )
    )


def _copy(nc, eng, out, in_):
    if eng is nc.scalar:
        eng.copy(out, in_)
    else:
        eng.tensor_copy(out, in_)


def _v_copies(nc, g, vt_ps, vT_sb, v_stg, v_sb):
    engines = [nc.vector, nc.scalar]
    for tl in range(4):
        r0 = (tl % 2) * 64
        c0 = (tl // 2) * 512
        _copy(nc, engines[(tl + 1) % 2], vT_sb[r0:r0 + 64, c0:c0 + 512],
              vt_ps[tl // 2][r0:r0 + 64, :])
    # batched v transposes: keys for (r0,c0) block are tb*512:(tb+1)*512 with
    # tb = 4g+tl, i.e. kc chunks 4*tb..4*tb+4.  The XBAR writes the transposed
    # block CONTIGUOUSLY -- a strided dest AP silently corrupts -- so land in
    # contiguous v_stg, then strided engine copies into v_sb (66-stride).
    for tl in range(4):
        r0 = (tl % 2) * 64
        c0 = (tl // 2) * 512
        nc.sync.dma_start_transpose(
            v_stg[:, g, 4 * tl:4 * tl + 4, :], vT_sb[r0:r0 + 64, c0:c0 + 512]
        )
    for hh in range(2):
        _copy(nc, engines[hh], v_sb[:, g * 16 + 8 * hh:g * 16 + 8 * hh + 8, 0:64],
              v_stg[:, g, 8 * hh:8 * hh + 8, :])


def _g0_copies(nc, kq_pair, vt_ps, qstage, qT3, kT3, vT_sb, v_stg, v_sb):
    engines = [nc.vector, nc.scalar]
    for tl in range(4):
        kq = kq_pair[tl // 2][:, tl % 2, :]
        # k at partitions 0:64 -> straight into kT3 (cast f32->bf16)
        _copy(nc, engines[tl % 2], kT3[:, 4 * tl:4 * tl + 4, :],
              kq[0:64, :].rearrange("p (kc f) -> p kc f", kc=4))
        # local queries: hop partitions 64:128 -> 0:64
        _copy(nc, engines[(tl + 1) % 2], qstage[64:128, tl, :], kq[64:128, :])
        nc.gpsimd.dma_start(out=qT3[:, tl, :], in_=qstage[64:128, tl, :])
    _v_copies(nc, 0, vt_ps, vT_sb, v_stg, v_sb)


def _g1_k_copies(nc, kq_g1, qstage, kT3):
    # kT3-producing copies FIRST on both engines: phase 2's pulled-ahead
    # LDWEIGHTS stall the PE in retire order if these land late.
    engines = [nc.vector, nc.scalar]
    for pair in range(2):
        tb_e = 4 + 2 * pair
        _copy(nc, engines[pair % 2], kT3[:, 4 * tb_e:4 * tb_e + 4, :],
              kq_g1[0:64, pair, :].rearrange("p (kc f) -> p kc f", kc=4))
        # odd-tb k sits at partitions 64:128 -> stage + hop down
        _copy(nc, engines[(pair + 1) % 2], qstage[64:128, 2 * pair, :],
              kq_g1[64:128, pair, :])
    for pair in range(2):
        tb_o = 5 + 2 * pair
        nc.gpsimd.dma_start(
            out=kT3[:, 4 * tb_o:4 * tb_o + 4, :],
            in_=qstage[64:128, 2 * pair, :].rearrange("p (kc f) -> p kc f", kc=4),
        )


def build_nc():
    nc = bacc.Bacc("TRN2", target_bir_lowering=False, debug=False, num_devices=8)

    xt_d = nc.dram_tensor("xt", [128, NCC, T], BF16, kind="ExternalInput")
    wkq_d = nc.dram_tensor("wkq", [128, NCC, 128], BF16, kind="ExternalInput")
    wv_d = nc.dram_tensor("wv", [128, NCC, H], BF16, kind="ExternalInput")
    nbias_d = nc.dram_tensor("nbias", [128, NKC], F32, kind="ExternalInput")
    maskq_d = nc.dram_tensor("maskq", [128, NTT], F32, kind="ExternalInput")
    out_d = nc.dram_tensor("out", [TL, H], F32, kind="ExternalOutput")
    dbg = {}
    if DEBUG:
        dbg["kT"] = nc.dram_tensor("dbg_kT", [64, NKC * 128], BF16, kind="ExternalOutput")
        dbg["qT"] = nc.dram_tensor("dbg_qT", [64, NQB * 512], BF16, kind="ExternalOutput")
        dbg["v"] = nc.dram_tensor("dbg_v", [128, NKC * 66], BF16, kind="ExternalOutput")

    with tile.TileContext(nc) as tc:
        with (
            tc.tile_pool(name="const", bufs=1) as const,
            tc.tile_pool(name="persist", bufs=1) as persist,
            tc.tile_pool(name="expp", bufs=4) as expp,
            tc.tile_pool(name="osb", bufs=1) as osb,
            tc.tile_pool(name="small", bufs=4) as small,
            tc.tile_pool(name="PS", bufs=2, space=bass.MemorySpace.PSUM) as PS,
            tc.tile_pool(name="PO", bufs=4, space=bass.MemorySpace.PSUM) as PO,
        ):
            _emit(nc, const, persist, expp, osb, small, PS, PO,
                  xt_d, wkq_d, wv_d, nbias_d, maskq_d, out_d, dbg)

    nc.compile()
    return nc


def _emit(nc, const, persist, expp, osb, small, PS, PO,
          xt_d, wkq_d, wv_d, nbias_d, maskq_d, out_d, dbg=None):
    # ---------------- constants / persistent tiles ----------------
    wkq_sb = const.tile([128, NCC, 128], BF16)
    wv_sb = const.tile([128, NCC, H], BF16)
    nbias_sb = const.tile([128, NKC], F32)
    maskq_sb = const.tile([128, NTT], F32)
    # wkq gates the first matmul -- put it first on the sync (HWDGE) queue
    nc.sync.dma_start(out=wkq_sb, in_=wkq_d.ap())
    nc.gpsimd.dma_start(out=wv_sb, in_=wv_d.ap())
    nc.gpsimd.dma_start(out=nbias_sb, in_=nbias_d.ap())
    nc.gpsimd.dma_start(out=maskq_sb, in_=maskq_d.ap())

    xT_sb = persist.tile([128, NCC, T], BF16)
    kT3 = persist.tile([64, NKC, 128], BF16)
    qT3 = persist.tile([64, NQB, 512], BF16)
    qstage = persist.tile([128, NQB, 512], BF16)
    v_sb = persist.tile([128, NKC, 66], BF16)       # [key, 65(+pad)]
    v_stg = persist.tile([128, 2, 16, 64], BF16)    # contiguous xbar dest
    vT_sb = persist.tile([128, 2 * 512], BF16)      # vT staging, parity rows
    oTT = persist.tile([128, NTT, 80], BF16)
    out_acc = persist.tile([128, NTT, H], F32)

    ones_sb = const.tile([128, NKC], BF16)
    nc.gpsimd.memset(ones_sb, 1.0)
    nc.gpsimd.tensor_copy(v_sb[:, :, 64], ones_sb)

    # ---------------- 1) x^T loads (host pre-transposed), g0 halves first --
    # First chunk split per-tl so the first projection matmul starts on a
    # 128KB transfer instead of waiting out a full 512KB one.
    for tl in range(4):
        nc.sync.dma_start(
            out=xT_sb[:, 0, tl * 512:(tl + 1) * 512],
            in_=xt_d.ap()[:, 0, tl * 512:(tl + 1) * 512],
        )
    for g in range(2):
        for cc in range(NCC):
            if g == 0 and cc == 0:
                continue
            nc.sync.dma_start(
                out=xT_sb[:, cc, g * TL:(g + 1) * TL],
                in_=xt_d.ap()[:, cc, g * TL:(g + 1) * TL],
            )

    # ---------------- 2) projections ----------------
    # g0: full [wk|wq] stationary (queries are local rows 0:2048)
    kq_pair = [PS.tile([128, 2, 512], F32, tag="s", name="kq")
               for _ in range(2)]
    vt_ps = [PO.tile([128, 512], F32, tag="o", name="vt") for _ in range(2)]
    for cc in range(NCC):
        first, last = cc == 0, cc == NCC - 1
        nc.tensor.ldweights(wkq_sb[:, cc, :])
        for tl in range(4):
            mm_noload(
                nc, kq_pair[tl // 2][:, tl % 2, :],
                wkq_sb[:, cc, :],
                xT_sb[:, cc, tl * 512:(tl + 1) * 512],
                start=first, stop=last,
            )
        for tl in range(4):
            r0 = (tl % 2) * 64
            nc.tensor.matmul(
                vt_ps[tl // 2][r0:r0 + 64, :],
                wv_sb[:, cc, :],
                xT_sb[:, cc, tl * 512:(tl + 1) * 512],
                start=first, stop=last,
            )
    _g0_copies(nc, kq_pair, vt_ps, qstage, qT3, kT3, vT_sb, v_stg, v_sb)

    # g1: k-only, col-packed -- tb pairs run concurrently in the PE array
    # (tb even -> array cols 0:64 / psum partitions 0:64, tb odd -> 64:128)
    kq_g1 = PS.tile([128, 2, 512], F32, tag="s", name="kq1")
    vt_ps1 = [PO.tile([128, 512], F32, tag="o", name="vt1") for _ in range(2)]
    # kq pass FIRST, v pass second: kq_g1's PSUM slot frees (via its copies)
    # while the v matmuls still run, so phase 2's kc0 score banks are ready
    # the moment g1 ends -- no PE hole at the phase boundary, HAM stays warm.
    for cc in range(NCC):
        first, last = cc == 0, cc == NCC - 1
        wk = wkq_sb[:, cc, 0:64]
        nc.tensor.ldweights(wk, tile_position=(0, 0))
        nc.tensor.ldweights(wk, tile_position=(0, 64))
        for pair in range(2):
            tb_e, tb_o = 4 + 2 * pair, 5 + 2 * pair
            mm_noload(
                nc, kq_g1[0:64, pair, :], wk,
                xT_sb[:, cc, tb_e * 512:(tb_e + 1) * 512],
                start=first, stop=last,
                tile_position=(0, 0), tile_size=(128, 64),
            )
            mm_noload(
                nc, kq_g1[64:128, pair, :], wk,
                xT_sb[:, cc, tb_o * 512:(tb_o + 1) * 512],
                start=first, stop=last,
                tile_position=(0, 64), tile_size=(128, 64),
            )
    _g1_k_copies(nc, kq_g1, qstage, kT3)

    # phase-2 kc0 scores+exp emitted HERE: keeps the PE/ACT queues full while
    # g1's v chain (matmuls -> copies -> transposes) drains, so the
    # phase-boundary sync resolves under queued work instead of idling the PE
    # (an idle boundary re-throttles HAM and the next ~8 kc run at 1.2 GHz).
    es0 = []
    nc.tensor.ldweights(kT3[:, 0, :])
    for grp in range(2):
        s = PS.tile([128, 2, 512], F32, tag="s", name="sT")
        for j in range(2):
            mm_noload(
                nc, s[:, j, :], kT3[:, 0, :], qT3[:, 2 * grp + j, :],
                start=True, stop=True,
            )
        e = expp.tile([128, 2, 512], BF16, name="exp")
        nc.scalar.activation(
            e, s, mybir.ActivationFunctionType.Exp,
            bias=nbias_sb[:, 0:1], scale=0.125,
        )
        es0.append(e)

    for cc in range(NCC):
        first, last = cc == 0, cc == NCC - 1
        for tl in range(4):
            tb = 4 + tl
            r0 = (tl % 2) * 64
            nc.tensor.matmul(
                vt_ps1[tl // 2][r0:r0 + 64, :],
                wv_sb[:, cc, :],
                xT_sb[:, cc, tb * 512:(tb + 1) * 512],
                start=first, stop=last,
            )
    _v_copies(nc, 1, vt_ps1, vT_sb, v_stg, v_sb)

    if DEBUG:
        nc.gpsimd.dma_start(out=dbg["kT"].ap(), in_=kT3.rearrange("p a f -> p (a f)"))
        nc.gpsimd.dma_start(out=dbg["qT"].ap(), in_=qT3.rearrange("p a f -> p (a f)"))
        nc.gpsimd.dma_start(out=dbg["v"].ap(), in_=v_sb.rearrange("p a f -> p (a f)"))

    # ---------------- 3) phase 2: kc-major, PV lags scores by one kc ------
    oT_ps = [PO.tile([128, 512], F32, tag="o", name=f"oT{i}")
             for i in range(NQB)]
    prev = (0, es0)  # kc0 scores/exp already emitted above
    for kc in range(1, NKC):
        es = []
        nc.tensor.ldweights(kT3[:, kc, :])
        for grp in range(2):
            s = PS.tile([128, 2, 512], F32, tag="s", name="sT")
            for j in range(2):
                mm_noload(
                    nc, s[:, j, :], kT3[:, kc, :], qT3[:, 2 * grp + j, :],
                    start=True, stop=True,
                )
            e = expp.tile([128, 2, 512], BF16, name="exp")
            nc.scalar.activation(
                e, s, mybir.ActivationFunctionType.Exp,
                bias=nbias_sb[:, kc:kc + 1], scale=0.125,
            )
            es.append(e)
        if prev is not None:
            pkc, pes = prev
            nc.tensor.ldweights(v_sb[:, pkc, 0:65])
            for qb in range(NQB):
                mm_noload(
                    nc, oT_ps[qb][0:65, :],
                    v_sb[:, pkc, 0:65], pes[qb // 2][:, qb % 2, :],
                    start=(pkc == 0), stop=(pkc == NKC - 1),
                )
        prev = (kc, es)
    pkc, pes = prev
    nc.tensor.ldweights(v_sb[:, pkc, 0:65])
    for qb in range(NQB):
        mm_noload(
            nc, oT_ps[qb][0:65, :],
            v_sb[:, pkc, 0:65], pes[qb // 2][:, qb % 2, :],
            start=(pkc == 0), stop=(pkc == NKC - 1),
        )

    # ---------------- 4) epilogue ----------------
    # Batched XBAR fold: oTT[p, 4*qb+b, j] = oTs[j, qb, b*128+p], i.e.
    # query = qb*512 + b*128 + p = tt*128 + p with tt = 4*qb+b.
    oTs = osb.tile([80, NQB, 512], BF16)
    engines = [nc.vector, nc.scalar]
    for qb in range(NQB):
        if qb % 2 == 0:
            nc.vector.tensor_copy(oTs[0:65, qb, :], oT_ps[qb][0:65, :])
        else:
            nc.scalar.copy(oTs[0:65, qb, :], oT_ps[qb][0:65, :])
    nc.sync.dma_start_transpose(
        oTT, oTs.rearrange("p a f -> p (a f)")
    )
    recip_all = small.tile([128, NTT], F32)
    scale_all = small.tile([128, NTT], F32)
    nc.vector.reciprocal(
        recip_all, oTT[:, :, 64:65].rearrange("p a one -> p (a one)")
    )
    nc.vector.tensor_tensor(
        scale_all, recip_all, maskq_sb, mybir.AluOpType.mult
    )
    out_dv = out_d.ap().rearrange("(n p) h -> p n h", p=128)
    for half in range(2):
        for i in range(8):
            tt = half * 8 + i
            sc = scale_all[:, tt:tt + 1]
            if i % 3 == 0:
                nc.vector.tensor_scalar(
                    out=out_acc[:, tt, :], in0=oTT[:, tt, 0:64],
                    scalar1=sc, scalar2=None, op0=mybir.AluOpType.mult,
                )
            elif i % 3 == 1:
                nc.scalar.mul(out_acc[:, tt, :], oTT[:, tt, 0:64], sc)
            else:
                nc.gpsimd.tensor_scalar(
                    out=out_acc[:, tt, :], in0=oTT[:, tt, 0:64],
                    scalar1=sc, scalar2=None, op0=mybir.AluOpType.mult,
                )
        nc.gpsimd.dma_start(
            out=out_dv[:, half * 8:(half + 1) * 8, :],
            in_=out_acc[:, half * 8:(half + 1) * 8, :],
        )


_NC_CACHE = None


def _get_nc():
    global _NC_CACHE
    if _NC_CACHE is None:
        _NC_CACHE = build_nc()
    return _NC_CACHE


def make_in_maps(x, padding_mask, Wk, Wq, Wv):
    x = np.asarray(x)
    padding_mask = np.asarray(padding_mask)

    def wt(w):  # [64,1024] -> [128, 8, 64]: wt[p, cc, h] = w[h, cc*128+p]
        return np.ascontiguousarray(
            np.asarray(w).T.reshape(NCC, 128, H).transpose(1, 0, 2)
        )

    wkt, wqt, wvt = wt(Wk), wt(Wq), wt(Wv)
    # stationary [wk | wq] -> psum rows 0:64 = k, 64:128 = q
    wkq = np.concatenate([wkt, wqt], axis=2).astype(ml_dtypes.bfloat16)
    wv = wvt.astype(ml_dtypes.bfloat16)

    in_maps = []
    for core in range(8):
        b, half = core // 2, core % 2
        # rotate keys so this core's queries are always rows 0:2048
        # (attention is permutation-invariant over keys when the key mask
        # is rotated identically)
        q0 = half * TL
        xb = np.roll(x[b], -q0, axis=0).astype(ml_dtypes.bfloat16)
        # host transpose: xt[p, cc, t] = xb[t, cc*128+p]
        xt = np.ascontiguousarray(
            xb.T.reshape(NCC, 128, T).transpose(1, 0, 2)
        )
        m = np.roll(padding_mask[b, 0].astype(np.float32), -q0)
        nbias = np.ascontiguousarray(
            (NEG * (1.0 - m)).reshape(NKC, 128).T
        )
        maskq = np.ascontiguousarray(m[0:TL].reshape(NTT, 128).T)
        in_maps.append({
            "xt": xt, "wkq": wkq, "wv": wv,
            "nbias": nbias, "maskq": maskq,
        })
    return in_maps


def kernel(x, padding_mask, Wk, Wq, Wv):
    nc = _get_nc()
    in_maps = make_in_maps(x, padding_mask, Wk, Wq, Wv)
    res = run_bass_kernel_spmd(nc, in_maps, core_ids=list(range(8)), trace=False)
    B = np.asarray(x).shape[0]
    out = np.empty((B, T, H), dtype=np.float32)
    for c in range(8):
        b, half = c // 2, c % 2
        out[b, half * TL:(half + 1) * TL, :] = res.results[c]["out"]
    return out


# revision 23
# speedup vs baseline: 1.2478x; 1.0384x over previous
"""Single-head attention kernel v5 for Trainium2 (8 NeuronCores, SPMD).

Problem: x[4,4096,1024] f32, padding_mask[4,1,4096] i32, Wk/Wq/Wv[64,1024] f32.
  k/q/v = x @ W.T ; wei = softmax(mask(q k^T / 8)) ; out = wei @ v  -> [4,4096,64]

Sharding: core c = (b = c//2, half = c%2). The host rotates x[b] (and the key
mask) so this core's 2048 queries are always rows 0:2048 -- attention is
permutation-invariant over keys when the key mask rotates identically.  Each
core computes k/v for all 4096 keys and q for its local half, returning
out[2048, 64].  No cross-core exchange.

v5 changes vs v4 (177us):
  - Stationary is [wk | wq]: k lands at PSUM partitions 0:64 -> direct engine
    copy into kT3 (no kstage staging + DMA hop).  q (g0 only) takes the small
    hop (qstage -> DMA down to partitions 0:64) instead.
  - v transposes batched: 4 XBAR transposes of [64,512] per group straight
    into v_sb slices (vs 16 of [64,128] + v_stg + gpsimd copy).  v4's 19us
    phase-1 stall was head-of-line blocking behind these on the sync queue.
  - x DMA split into per-(cc, half) chunks, g0 halves first, so g0
    projections start after ~1.5us of DMA.
  - Epilogue: one batched oT transpose; recip+mask fused into one scale
    vector; per-tt multiplies split across vector/scalar engines.
"""

import sys

if "/opt/trn_rl_repo" not in sys.path:
    sys.path.insert(0, "/opt/trn_rl_repo")

import numpy as np
import ml_dtypes

import concourse.bass as bass
import concourse.mybir as mybir
import concourse.tile as tile
from concourse import bacc
from concourse.bass_utils import run_bass_kernel_spmd

F32 = mybir.dt.float32
BF16 = mybir.dt.bfloat16

DEBUG = False

T = 4096
TL = 2048
C = 1024
H = 64
NCC = 8
NKC = 32
NQB = 4
NTT = TL // 128   # 16
NEG = -1.0e5


def mm_noload(nc, out, lhsT, rhs, start=True, stop=True, tile_position=None,
              tile_size=(128, 128)):
    te = nc.tensor
    keep = {0}
    ifmap_ap = te.lower_ap(rhs.opt(keep), opt=False)
    weights_ap = te.lower_ap(lhsT.opt(keep), opt=False, for_matmul_weights=True)
    out_ap = te.lower_ap(out)
    if tile_position is None:
        tile_position = (rhs.base_partition(), out.base_partition())
    return te.add_instruction(
        mybir.InstMatmult(
            name=f"I-{nc.next_id()}",
            replication_resolution=0,
            replication_shift_amnt=0,
            replication_num_rows=0,
            start_tensor_calc=start,
            stop_tensor_calc=stop,
            ins=[ifmap_ap, weights_ap],
            outs=[out_ap],
            perf_mode=None,
            is_transpose=None,
            tile_position=tile_position,
            tile_size=tile_size,
            ldweights=False,
        )
    )


def _copy(nc, eng, out, in_):
    if eng is nc.scalar:
        eng.copy(out, in_)
    else:
        eng.tensor_copy(out, in_)


def _v_copies(nc, g, vt_ps, vT_sb, v_stg, v_sb):
    engines = [nc.vector, nc.scalar]
    for tl in range(4):
        r0 = (tl % 2) * 64
        c0 = (tl // 2) * 512
        _copy(nc, engines[(tl + 1) % 2], vT_sb[r0:r0 + 64, c0:c0 + 512],
              vt_ps[tl // 2][r0:r0 + 64, :])
    # batched v transposes: keys for (r0,c0) block are tb*512:(tb+1)*512 with
    # tb = 4g+tl, i.e. kc chunks 4*tb..4*tb+4.  The XBAR writes the transposed
    # block CONTIGUOUSLY -- a strided dest AP silently corrupts -- so land in
    # contiguous v_stg, then strided engine copies into v_sb (66-stride).
    for tl in range(4):
        r0 = (tl % 2) * 64
        c0 = (tl // 2) * 512
        nc.sync.dma_start_transpose(
            v_stg[:, g, 4 * tl:4 * tl + 4, :], vT_sb[r0:r0 + 64, c0:c0 + 512]
        )
    for hh in range(2):
        _copy(nc, engines[hh], v_sb[:, g * 16 + 8 * hh:g * 16 + 8 * hh + 8, 0:64],
              v_stg[:, g, 8 * hh:8 * hh + 8, :])


def _g0_copies(nc, kq_pair, vt_ps, qstage, qT3, kT3, vT_sb, v_stg, v_sb):
    engines = [nc.vector, nc.scalar]
    for tl in range(4):
        kq = kq_pair[tl // 2][:, tl % 2, :]
        # k at partitions 0:64 -> straight into kT3 (cast f32->bf16)
        _copy(nc, engines[tl % 2], kT3[:, 4 * tl:4 * tl + 4, :],
              kq[0:64, :].rearrange("p (kc f) -> p kc f", kc=4))
        # local queries: hop partitions 64:128 -> 0:64
        _copy(nc, engines[(tl + 1) % 2], qstage[64:128, tl, :], kq[64:128, :])
        nc.gpsimd.dma_start(out=qT3[:, tl, :], in_=qstage[64:128, tl, :])
    _v_copies(nc, 0, vt_ps, vT_sb, v_stg, v_sb)


def _g1_k_copies(nc, kq_g1, qstage, kT3):
    # kT3-producing copies FIRST on both engines: phase 2's pulled-ahead
    # LDWEIGHTS stall the PE in retire order if these land late.
    engines = [nc.vector, nc.scalar]
    for pair in range(2):
        tb_e = 4 + 2 * pair
        _copy(nc, engines[pair % 2], kT3[:, 4 * tb_e:4 * tb_e + 4, :],
              kq_g1[0:64, pair, :].rearrange("p (kc f) -> p kc f", kc=4))
        # odd-tb k sits at partitions 64:128 -> stage + hop down
        _copy(nc, engines[(pair + 1) % 2], qstage[64:128, 2 * pair, :],
              kq_g1[64:128, pair, :])
    for pair in range(2):
        tb_o = 5 + 2 * pair
        nc.gpsimd.dma_start(
            out=kT3[:, 4 * tb_o:4 * tb_o + 4, :],
            in_=qstage[64:128, 2 * pair, :].rearrange("p (kc f) -> p kc f", kc=4),
        )


def build_nc():
    nc = bacc.Bacc("TRN2", target_bir_lowering=False, debug=False, num_devices=8)

    xt_d = nc.dram_tensor("xt", [128, NCC, T], BF16, kind="ExternalInput")
    wkq_d = nc.dram_tensor("wkq", [128, NCC, 128], BF16, kind="ExternalInput")
    wv_d = nc.dram_tensor("wv", [128, NCC, H], BF16, kind="ExternalInput")
    nbias_d = nc.dram_tensor("nbias", [128, NKC], F32, kind="ExternalInput")
    maskq_d = nc.dram_tensor("maskq", [128, NTT], F32, kind="ExternalInput")
    out_d = nc.dram_tensor("out", [TL, H], F32, kind="ExternalOutput")
    dbg = {}
    if DEBUG:
        dbg["kT"] = nc.dram_tensor("dbg_kT", [64, NKC * 128], BF16, kind="ExternalOutput")
        dbg["qT"] = nc.dram_tensor("dbg_qT", [64, NQB * 512], BF16, kind="ExternalOutput")
        dbg["v"] = nc.dram_tensor("dbg_v", [128, NKC * 66], BF16, kind="ExternalOutput")

    with tile.TileContext(nc) as tc:
        with (
            tc.tile_pool(name="const", bufs=1) as const,
            tc.tile_pool(name="persist", bufs=1) as persist,
            tc.tile_pool(name="expp", bufs=4) as expp,
            tc.tile_pool(name="osb", bufs=1) as osb,
            tc.tile_pool(name="small", bufs=4) as small,
            tc.tile_pool(name="PS", bufs=2, space=bass.MemorySpace.PSUM) as PS,
            tc.tile_pool(name="PO", bufs=4, space=bass.MemorySpace.PSUM) as PO,
        ):
            _emit(nc, const, persist, expp, osb, small, PS, PO,
                  xt_d, wkq_d, wv_d, nbias_d, maskq_d, out_d, dbg)

    nc.compile()
    return nc


def _emit(nc, const, persist, expp, osb, small, PS, PO,
          xt_d, wkq_d, wv_d, nbias_d, maskq_d, out_d, dbg=None):
    # ---------------- constants / persistent tiles ----------------
    wkq_sb = const.tile([128, NCC, 128], BF16)
    wv_sb = const.tile([128, NCC, H], BF16)
    nbias_sb = const.tile([128, NKC], F32)
    maskq_sb = const.tile([128, NTT], F32)
    # wkq gates the first matmul -- gpsimd queue runs it in parallel with
    # the first x chunk on the sync queue
    nc.gpsimd.dma_start(out=wkq_sb, in_=wkq_d.ap())
    nc.gpsimd.dma_start(out=wv_sb, in_=wv_d.ap())
    nc.gpsimd.dma_start(out=nbias_sb, in_=nbias_d.ap())
    nc.gpsimd.dma_start(out=maskq_sb, in_=maskq_d.ap())

    xT_sb = persist.tile([128, NCC, T], BF16)
    kT3 = persist.tile([64, NKC, 128], BF16)
    qT3 = persist.tile([64, NQB, 512], BF16)
    qstage = persist.tile([128, NQB, 512], BF16)
    v_sb = persist.tile([128, NKC, 66], BF16)       # [key, 65(+pad)]
    v_stg = persist.tile([128, 2, 16, 64], BF16)    # contiguous xbar dest
    vT_sb = persist.tile([128, 2 * 512], BF16)      # vT staging, parity rows
    oTT = persist.tile([128, NTT, 80], BF16)
    out_acc = persist.tile([128, NTT, H], F32)

    ones_sb = const.tile([128, NKC], BF16)
    nc.gpsimd.memset(ones_sb, 1.0)
    nc.gpsimd.tensor_copy(v_sb[:, :, 64], ones_sb)

    # ---------------- 1) x^T loads (host pre-transposed), g0 halves first --
    # First chunk split per-tl so the first projection matmul starts on a
    # 128KB transfer instead of waiting out a full 512KB one.
    for tl in range(4):
        nc.sync.dma_start(
            out=xT_sb[:, 0, tl * 512:(tl + 1) * 512],
            in_=xt_d.ap()[:, 0, tl * 512:(tl + 1) * 512],
        )
    for g in range(2):
        for cc in range(NCC):
            if g == 0 and cc == 0:
                continue
            nc.sync.dma_start(
                out=xT_sb[:, cc, g * TL:(g + 1) * TL],
                in_=xt_d.ap()[:, cc, g * TL:(g + 1) * TL],
            )

    # ---------------- 2) projections ----------------
    # g0: full [wk|wq] stationary (queries are local rows 0:2048)
    kq_pair = [PS.tile([128, 2, 512], F32, tag="s", name="kq")
               for _ in range(2)]
    vt_ps = [PO.tile([128, 512], F32, tag="o", name="vt") for _ in range(2)]
    for cc in range(NCC):
        first, last = cc == 0, cc == NCC - 1
        nc.tensor.ldweights(wkq_sb[:, cc, :])
        for tl in range(4):
            mm_noload(
                nc, kq_pair[tl // 2][:, tl % 2, :],
                wkq_sb[:, cc, :],
                xT_sb[:, cc, tl * 512:(tl + 1) * 512],
                start=first, stop=last,
            )
        for tl in range(4):
            r0 = (tl % 2) * 64
            nc.tensor.matmul(
                vt_ps[tl // 2][r0:r0 + 64, :],
                wv_sb[:, cc, :],
                xT_sb[:, cc, tl * 512:(tl + 1) * 512],
                start=first, stop=last,
            )
    _g0_copies(nc, kq_pair, vt_ps, qstage, qT3, kT3, vT_sb, v_stg, v_sb)

    # g1: k-only, col-packed -- tb pairs run concurrently in the PE array
    # (tb even -> array cols 0:64 / psum partitions 0:64, tb odd -> 64:128)
    kq_g1 = PS.tile([128, 2, 512], F32, tag="s", name="kq1")
    vt_ps1 = [PO.tile([128, 512], F32, tag="o", name="vt1") for _ in range(2)]
    # kq pass FIRST, v pass second: kq_g1's PSUM slot frees (via its copies)
    # while the v matmuls still run, so phase 2's kc0 score banks are ready
    # the moment g1 ends -- no PE hole at the phase boundary, HAM stays warm.
    for cc in range(NCC):
        first, last = cc == 0, cc == NCC - 1
        wk = wkq_sb[:, cc, 0:64]
        nc.tensor.ldweights(wk, tile_position=(0, 0))
        nc.tensor.ldweights(wk, tile_position=(0, 64))
        for pair in range(2):
            tb_e, tb_o = 4 + 2 * pair, 5 + 2 * pair
            mm_noload(
                nc, kq_g1[0:64, pair, :], wk,
                xT_sb[:, cc, tb_e * 512:(tb_e + 1) * 512],
                start=first, stop=last,
                tile_position=(0, 0), tile_size=(128, 64),
            )
            mm_noload(
                nc, kq_g1[64:128, pair, :], wk,
                xT_sb[:, cc, tb_o * 512:(tb_o + 1) * 512],
                start=first, stop=last,
                tile_position=(0, 64), tile_size=(128, 64),
            )
    _g1_k_copies(nc, kq_g1, qstage, kT3)

    # phase-2 kc0 scores+exp emitted HERE: keeps the PE/ACT queues full while
    # g1's v chain (matmuls -> copies -> transposes) drains, so the
    # phase-boundary sync resolves under queued work instead of idling the PE
    # (an idle boundary re-throttles HAM and the next ~8 kc run at 1.2 GHz).
    es0 = []
    nc.tensor.ldweights(kT3[:, 0, :])
    for grp in range(2):
        s = PS.tile([128, 2, 512], F32, tag="s", name="sT")
        for j in range(2):
            mm_noload(
                nc, s[:, j, :], kT3[:, 0, :], qT3[:, 2 * grp + j, :],
                start=True, stop=True,
            )
        e = expp.tile([128, 2, 512], BF16, name="exp")
        nc.scalar.activation(
            e, s, mybir.ActivationFunctionType.Exp,
            bias=nbias_sb[:, 0:1], scale=0.125,
        )
        es0.append(e)

    for cc in range(NCC):
        first, last = cc == 0, cc == NCC - 1
        for tl in range(4):
            tb = 4 + tl
            r0 = (tl % 2) * 64
            nc.tensor.matmul(
                vt_ps1[tl // 2][r0:r0 + 64, :],
                wv_sb[:, cc, :],
                xT_sb[:, cc, tb * 512:(tb + 1) * 512],
                start=first, stop=last,
            )
    _v_copies(nc, 1, vt_ps1, vT_sb, v_stg, v_sb)

    if DEBUG:
        nc.gpsimd.dma_start(out=dbg["kT"].ap(), in_=kT3.rearrange("p a f -> p (a f)"))
        nc.gpsimd.dma_start(out=dbg["qT"].ap(), in_=qT3.rearrange("p a f -> p (a f)"))
        nc.gpsimd.dma_start(out=dbg["v"].ap(), in_=v_sb.rearrange("p a f -> p (a f)"))

    # ---------------- 3) phase 2: kc-major, PV lags scores by one kc ------
    oT_ps = [PO.tile([128, 512], F32, tag="o", name=f"oT{i}")
             for i in range(NQB)]
    prev = (0, es0)  # kc0 scores/exp already emitted above
    for kc in range(1, NKC):
        es = []
        nc.tensor.ldweights(kT3[:, kc, :])
        for grp in range(2):
            s = PS.tile([128, 2, 512], F32, tag="s", name="sT")
            for j in range(2):
                mm_noload(
                    nc, s[:, j, :], kT3[:, kc, :], qT3[:, 2 * grp + j, :],
                    start=True, stop=True,
                )
            e = expp.tile([128, 2, 512], BF16, name="exp")
            nc.scalar.activation(
                e, s, mybir.ActivationFunctionType.Exp,
                bias=nbias_sb[:, kc:kc + 1], scale=0.125,
            )
            es.append(e)
        if prev is not None:
            pkc, pes = prev
            nc.tensor.ldweights(v_sb[:, pkc, 0:65])
            for qb in range(NQB):
                mm_noload(
                    nc, oT_ps[qb][0:65, :],
                    v_sb[:, pkc, 0:65], pes[qb // 2][:, qb % 2, :],
                    start=(pkc == 0), stop=(pkc == NKC - 1),
                )
        prev = (kc, es)
    # tail PVs for the last kc, interleaved with the per-qb epilogue copies
    # so the XBAR fold of the first half starts ~2us earlier.
    pkc, pes = prev
    nc.tensor.ldweights(v_sb[:, pkc, 0:65])
    oTs = osb.tile([80, NQB, 512], BF16)

    def tail_pv(qb):
        mm_noload(
            nc, oT_ps[qb][0:65, :],
            v_sb[:, pkc, 0:65], pes[qb // 2][:, qb % 2, :],
            start=(pkc == 0), stop=(pkc == NKC - 1),
        )

    tail_pv(0)
    tail_pv(1)
    nc.vector.tensor_copy(oTs[0:65, 0, :], oT_ps[0][0:65, :])
    tail_pv(2)
    nc.scalar.copy(oTs[0:65, 1, :], oT_ps[1][0:65, :])
    tail_pv(3)
    nc.sync.dma_start_transpose(
        oTT[:, 0:8, :], oTs[:, 0:2, :].rearrange("p a f -> p (a f)")
    )
    nc.vector.tensor_copy(oTs[0:65, 2, :], oT_ps[2][0:65, :])
    nc.scalar.copy(oTs[0:65, 3, :], oT_ps[3][0:65, :])
    nc.sync.dma_start_transpose(
        oTT[:, 8:16, :], oTs[:, 2:4, :].rearrange("p a f -> p (a f)")
    )

    # ---------------- 4) epilogue ----------------
    # oTT[p, 4*qb+b, j] = oTs[j, qb, b*128+p]: query = tt*128+p, tt = 4*qb+b.
    out_dv = out_d.ap().rearrange("(n p) h -> p n h", p=128)
    for half in range(2):
        recip_h = small.tile([128, 8], F32, name=f"recip{half}")
        scale_h = small.tile([128, 8], F32, name=f"scale{half}")
        nc.vector.reciprocal(
            recip_h,
            oTT[:, half * 8:(half + 1) * 8, 64:65].rearrange("p a one -> p (a one)"),
        )
        nc.vector.tensor_tensor(
            scale_h, recip_h, maskq_sb[:, half * 8:(half + 1) * 8],
            mybir.AluOpType.mult,
        )
        for i in range(8):
            tt = half * 8 + i
            sc = scale_h[:, i:i + 1]
            if i % 3 == 0:
                nc.vector.tensor_scalar(
                    out=out_acc[:, tt, :], in0=oTT[:, tt, 0:64],
                    scalar1=sc, scalar2=None, op0=mybir.AluOpType.mult,
                )
            elif i % 3 == 1:
                nc.scalar.mul(out_acc[:, tt, :], oTT[:, tt, 0:64], sc)
            else:
                nc.gpsimd.tensor_scalar(
                    out=out_acc[:, tt, :], in0=oTT[:, tt, 0:64],
                    scalar1=sc, scalar2=None, op0=mybir.AluOpType.mult,
                )
        nc.gpsimd.dma_start(
            out=out_dv[:, half * 8:(half + 1) * 8, :],
            in_=out_acc[:, half * 8:(half + 1) * 8, :],
        )


_NC_CACHE = None


def _get_nc():
    global _NC_CACHE
    if _NC_CACHE is None:
        _NC_CACHE = build_nc()
    return _NC_CACHE


def make_in_maps(x, padding_mask, Wk, Wq, Wv):
    x = np.asarray(x)
    padding_mask = np.asarray(padding_mask)

    def wt(w):  # [64,1024] -> [128, 8, 64]: wt[p, cc, h] = w[h, cc*128+p]
        return np.ascontiguousarray(
            np.asarray(w).T.reshape(NCC, 128, H).transpose(1, 0, 2)
        )

    wkt, wqt, wvt = wt(Wk), wt(Wq), wt(Wv)
    # stationary [wk | wq] -> psum rows 0:64 = k, 64:128 = q
    wkq = np.concatenate([wkt, wqt], axis=2).astype(ml_dtypes.bfloat16)
    wv = wvt.astype(ml_dtypes.bfloat16)

    in_maps = []
    for core in range(8):
        b, half = core // 2, core % 2
        # rotate keys so this core's queries are always rows 0:2048
        # (attention is permutation-invariant over keys when the key mask
        # is rotated identically)
        q0 = half * TL
        xb = np.roll(x[b], -q0, axis=0).astype(ml_dtypes.bfloat16)
        # host transpose: xt[p, cc, t] = xb[t, cc*128+p]
        xt = np.ascontiguousarray(
            xb.T.reshape(NCC, 128, T).transpose(1, 0, 2)
        )
        m = np.roll(padding_mask[b, 0].astype(np.float32), -q0)
        nbias = np.ascontiguousarray(
            (NEG * (1.0 - m)).reshape(NKC, 128).T
        )
        maskq = np.ascontiguousarray(m[0:TL].reshape(NTT, 128).T)
        in_maps.append({
            "xt": xt, "wkq": wkq, "wv": wv,
            "nbias": nbias, "maskq": maskq,
        })
    return in_maps


def kernel(x, padding_mask, Wk, Wq, Wv):
    nc = _get_nc()
    in_maps = make_in_maps(x, padding_mask, Wk, Wq, Wv)
    res = run_bass_kernel_spmd(nc, in_maps, core_ids=list(range(8)), trace=False)
    B = np.asarray(x).shape[0]
    out = np.empty((B, T, H), dtype=np.float32)
    for c in range(8):
        b, half = c // 2, c % 2
        out[b, half * TL:(half + 1) * TL, :] = res.results[c]["out"]
    return out
